# revision 6
# baseline (speedup 1.0000x reference)
"""Cosine-similarity scorer (CosScorer) as a Bass/Tile kernel on 8 TRN2 NeuronCores.

Problem: xs_pad (8, 4096, 512) f32, spk_emb (8, 256, 512) f32
         -> scores (8, 4096, 256) f32
         scores[b, t, s] = <xs[b,t], spk[b,s]> / (||xs[b,t]|| * ||spk[b,s]||)

Sharding: data-parallel over B -- core b computes batch b.

v3 design (from v2 trace analysis: exec window = first-user-inst ..
end-of-teardown; PE issue rate 259ns per 512-col matmul; DVE psum-source
ops run 1x; all traffic on one queue already hits ~430GB/s):
- y is NOT normalized on device. inv_y (per output-partition scalar) folds
  into the PSUM evacuation via scalar_tensor_tensor:
  ob = (po * inv_y[P,1]) * inv_x. Kills the whole y-norm chain (2 ACT,
  2 MM, 1 DVE mul) and takes y off the GEMM critical path.
- inv_y computed from a second host-staged copy of y (s-major) via
  ACT square with accum_out (free-dim reduce) -> rsqrt. No transpose.
- x-norm: squares split Scalar (c0:2 fused) / DVE (c2, c3); DVE pair-adds
  reduce 4 channels -> 1, so ONE ones-matmul per 512-t tile (was 2).
- Evacuation split DVE (s-chunk 0) / GpSimd (s-chunk 1).
- Input DMAs split per contraction chunk (16 x ~256KB) so squares start
  as soon as each chunk lands; warmup MMs cover the HAM ramp.
- PSUM: po pool [128,2,512] f32 bufs=3 (6 banks) + pn [128,512] bufs=2.
- Per-engine emission order hand-scheduled (see SCHEDULE below) so no
  engine FIFO head-blocks the PE.
"""

import numpy as np

import concourse.bacc as bacc
import concourse.tile as tile
from concourse import mybir
from concourse import bass_utils
from concourse.alu_op_type import AluOpType

B, T, D, S = 8, 4096, 512, 256
P = 128            # SBUF partitions
DC = D // P        # 4 contraction chunks
TT = 512           # t-tile width (psum bank = 512 f32)
NG = 4             # x pieces (DMA + norm granularity), 1024 t each
GW = T // NG       # 1024 t per piece
F32 = mybir.dt.float32
BF16 = mybir.dt.bfloat16
ACT = mybir.ActivationFunctionType

_NC_CACHE = {}


def _raw_rsqrt(nc, out, in_):
    """ACT Rsqrt via raw InstActivation.

    bass's activation() refuses Rsqrt citing accuracy; measured 3.9e-5
    max-rel on our norm^2 range -- far inside the 2e-2 budget -- and it
    keeps the norm chain on one ACT table (reciprocal_sqrt_and_small
    holds square + reciprocal_sqrt).
    """
    e = nc.scalar
    bias = nc.const_aps.scalar_like(0.0, in_)
    ins = [e.lower_ap(in_), e.lower_ap(bias),
           mybir.ImmediateValue(dtype=mybir.dt.float32, value=1.0),
           mybir.ImmediateValue(dtype=mybir.dt.float32, value=0.0)]
    return e.add_instruction(mybir.InstActivation(
        name=nc.get_next_instruction_name(),
        func=ACT.Rsqrt,
        ins=ins, outs=[e.lower_ap(out)]))


def build_nc():
    nc = bacc.Bacc(trn_type="TRN2", debug=False)

    # x piece g: [128, 4096] bf16, partition-major ([p, (c t')]) so each
    # (partition, chunk) row is one contiguous 2KB DMA run.
    xg = [
        nc.dram_tensor(f"xg{g}", [P, DC * GW], BF16, kind="ExternalInput")
        for g in range(NG)
    ]
    # yTp[p, (c s)] = y^T[c*128+p, s]  (stationary layout, 2KB rows)
    yTp = nc.dram_tensor("yTp", [P, DC * S], BF16, kind="ExternalInput")
    # ysm[p, (j d)] = y[j*128+p, d]   (s-major copy, norms only)
    ysm = nc.dram_tensor("ysm", [P, 2 * D], BF16, kind="ExternalInput")
    # out staged [p, piece, s-chunk, t'] so each store descriptor is a
    # contiguous 2KB partition run
    outS = nc.dram_tensor("outS", [P, NG, 2, GW], BF16,
                          kind="ExternalOutput")

    with tile.TileContext(nc) as tc:
        with (
            tc.tile_pool(name="const", bufs=1) as const_pool,
            tc.tile_pool(name="xall", bufs=1) as xall_pool,
            tc.tile_pool(name="ypool", bufs=1) as ypool,
            tc.tile_pool(name="xsq", bufs=2) as xsq_pool,
            tc.tile_pool(name="spp", bufs=2) as sp_pool,
            tc.tile_pool(name="ssum", bufs=2) as ssum_pool,
            tc.tile_pool(name="invp", bufs=4) as inv_pool,
            tc.tile_pool(name="outp", bufs=3) as out_pool,
            tc.tile_pool(name="psum_n", bufs=4, space="PSUM") as psn_pool,
            tc.tile_pool(name="psum_o", bufs=2, space="PSUM") as pso_pool,
        ):
            # ---- input DMAs up front (sync queue, FIFO): y copies first
            # (small, gate the y-norm + first LDWEIGHTS), then x pieces
            # split per contraction chunk so squares start per-chunk ----
            ysb = ypool.tile([P, 2, D], BF16)
            nc.scalar.dma_start(
                out=ysb, in_=ysm.ap().rearrange("p (j d) -> p j d", j=2))
            ytb = ypool.tile([P, DC, S], BF16)
            nc.scalar.dma_start(
                out=ytb, in_=yTp.ap().rearrange("p (c s) -> p c s", c=DC))
            x_all = xall_pool.tile([P, NG, DC, GW], BF16)
            for g in range(NG):
                xv = xg[g].ap().rearrange("p (c t) -> p c t", c=DC)
                if g < 2:
                    # per-chunk so the first GEMM/square starts per-chunk
                    for c in range(DC):
                        nc.sync.dma_start(out=x_all[:, g, c, :],
                                          in_=xv[:, c, :])
                else:
                    nc.sync.dma_start(out=x_all[:, g, :, :], in_=xv)

            ones = const_pool.tile([P, P], BF16)
            nc.vector.memset(ones, 1.0)
            warm = const_pool.tile([P, TT], BF16)
            nc.vector.memset(warm, 0.0)

            # ---- y-norm chain (Scalar only, off critical path):
            # sumsq via ACT square with free-dim accumulator, then rsqrt.
            ysq_scr = ypool.tile([P, D], BF16)
            ysum = ypool.tile([P, 2], F32)
            for j in range(2):
                nc.scalar.activation(
                    out=ysq_scr, in_=ysb[:, j, :], func=ACT.Square,
                    accum_out=ysum[:, j:j + 1])
            inv_y = ypool.tile([P, 2], F32)
            _raw_rsqrt(nc, inv_y, ysum)

            # ---- PE warmup: cover the HAM ramp while first DMAs fly ----
            wps = psn_pool.tile([P, TT], F32, tag="n")
            for _ in range(3):
                nc.tensor.matmul(wps, ones, warm, start=True, stop=True)

            # ---- per-piece pipeline pieces (emission helpers) ----
            xsq = {}
            ssum = {}
            pn = {}
            inv = {}
            po = {}
            ob = {}

            def emit_sq_scalar(g):
                # Scalar: fused square of chunks 0:3; GpSimd: chunk 3
                # (GpSimd cannot touch PSUM, so evacs all live on DVE and
                # DVE does no squares)
                xsq[g] = xsq_pool.tile([P, DC, GW], BF16, tag="xsq", name=f"xsq{g}")
                nc.scalar.square(xsq[g][:, 0:2, :], x_all[:, g, 0:2, :])
                nc.gpsimd.tensor_mul(
                    xsq[g][:, 3, :], x_all[:, g, 3, :], x_all[:, g, 3, :])

            def emit_sq_dve_adds(g):
                # DVE: square c2, then reduce 4 channels -> 1
                nc.vector.tensor_mul(
                    xsq[g][:, 2, :], x_all[:, g, 2, :], x_all[:, g, 2, :])
                sp = sp_pool.tile([P, 2, GW], BF16, tag="sp", name=f"sp{g}")
                nc.vector.tensor_add(sp, xsq[g][:, 0:2, :], xsq[g][:, 2:4, :])
                ssum[g] = ssum_pool.tile([P, GW], BF16, tag="ss", name=f"ss{g}")
                nc.vector.tensor_add(ssum[g], sp[:, 0, :], sp[:, 1, :])

            def emit_norm_mm(g):
                # PE: one ones-matmul per 512-t half -> ||x||^2 replicated
                pn[g] = [psn_pool.tile([P, TT], F32, tag="n", name=f"pn{g}_{h}")
                         for h in range(2)]
                for h in range(2):
                    nc.tensor.matmul(pn[g][h], ones,
                                     ssum[g][:, h * TT:(h + 1) * TT],
                                     start=True, stop=True)

            def emit_rsqrt(g):
                inv[g] = [inv_pool.tile([P, TT], F32, tag="inv", name=f"inv{g}_{h}")
                          for h in range(2)]
                for h in range(2):
                    _raw_rsqrt(nc, inv[g][h], pn[g][h])

            def emit_gemm(g, h):
                # PE: 2 s-chunks x 4 c-chunks, raw-y stationary
                if h == 0:
                    po[g] = [None, None]
                    ob[g] = out_pool.tile([P, 2, GW], BF16, tag="ob",
                                          name=f"ob{g}")
                pt = pso_pool.tile([P, 2, TT], F32, tag="o", name=f"po{g}_{h}")
                po[g][h] = pt
                for s in range(2):
                    for c in range(DC):
                        nc.tensor.matmul(
                            pt[:, s, :],
                            ytb[:, c, s * P:(s + 1) * P],
                            x_all[:, g, c, h * TT:(h + 1) * TT],
                            start=(c == 0), stop=(c == DC - 1),
                        )

            def emit_evac(g, h):
                # ob = (po * inv_y) * inv_x ; DVE only (PSUM source)
                for s, eng in ((0, nc.vector), (1, nc.vector)):
                    eng.scalar_tensor_tensor(
                        out=ob[g][:, s, h * TT:(h + 1) * TT],
                        in0=po[g][h][:, s, :],
                        scalar=inv_y[:, s:s + 1],
                        in1=inv[g][h],
                        op0=AluOpType.mult, op1=AluOpType.mult,
                    )

            def emit_store(g):
                nc.sync.dma_start(out=outS.ap()[:, g, :, :], in_=ob[g])

            # ---- SCHEDULE (per-engine order is what matters; see
            # docstring). PE: G(0),N(0),G1a,N1,G1b,G2a,N2,G2b,N3,G3. ----
            emit_sq_scalar(0)
            emit_sq_dve_adds(0)
            emit_gemm(0, 0)
            emit_gemm(0, 1)
            emit_norm_mm(0)

            emit_sq_scalar(1)          # scalar: sq(1) before rsqrt(0)
            emit_sq_dve_adds(1)
            emit_rsqrt(0)
            emit_evac(0, 0)
            emit_evac(0, 1)
            emit_store(0)

            emit_gemm(1, 0)
            emit_norm_mm(1)
            emit_gemm(1, 1)

            emit_sq_scalar(2)
            emit_sq_dve_adds(2)
            emit_rsqrt(1)
            emit_evac(1, 0)
            emit_evac(1, 1)
            emit_store(1)

            emit_gemm(2, 0)
            emit_norm_mm(2)
            emit_gemm(2, 1)

            emit_sq_scalar(3)
            emit_sq_dve_adds(3)
            emit_norm_mm(3)            # PE: N(3) before G(3) (data ready)
            emit_rsqrt(2)
            emit_rsqrt(3)
            emit_evac(2, 0)
            emit_evac(2, 1)
            emit_store(2)

            emit_gemm(3, 0)
            emit_gemm(3, 1)
            emit_evac(3, 0)
            emit_evac(3, 1)
            emit_store(3)

    nc.compile()
    return nc


def _get_nc():
    if "nc" not in _NC_CACHE:
        _NC_CACHE["nc"] = build_nc()
    return _NC_CACHE["nc"]


def _stage_inputs(xs, sp):
    """Host staging: bf16, d-major transpose, piece-major x layout."""
    import ml_dtypes

    xs = np.asarray(xs, dtype=np.float32)
    sp = np.asarray(sp, dtype=np.float32)
    in_maps = []
    for b in range(B):
        xT = np.ascontiguousarray(xs[b].T).astype(ml_dtypes.bfloat16)
        # [512, 4096] -> [c, p, g, t'] -> piece g: [p, (c t')]
        x4 = xT.reshape(DC, P, NG, GW)
        m = {
            f"xg{g}": np.ascontiguousarray(
                x4[:, :, g, :].transpose(1, 0, 2)
            ).reshape(P, DC * GW)
            for g in range(NG)
        }
        yt = np.ascontiguousarray(sp[b].T).astype(ml_dtypes.bfloat16)
        # yTp[p, c*S+s] = y^T[c*128+p, s]
        m["yTp"] = np.ascontiguousarray(
            yt.reshape(DC, P, S).transpose(1, 0, 2)).reshape(P, DC * S)
        # ysm[p, j*D+d] = y[j*128+p, d]
        ysb = sp[b].astype(ml_dtypes.bfloat16)
        m["ysm"] = np.ascontiguousarray(
            ysb.reshape(2, P, D).transpose(1, 0, 2)).reshape(P, 2 * D)
        in_maps.append(m)
    return in_maps


def run(inputs, **spmd_kwargs):
    """Run on 8 cores; returns (full output, BassKernelResults)."""
    xs = inputs["xs_pad"]
    sp = inputs["spk_emb"]
    nc = _get_nc()
    in_maps = _stage_inputs(xs, sp)
    res = bass_utils.run_bass_kernel_spmd(
        nc, in_maps, core_ids=list(range(B)), **spmd_kwargs
    )
    out = np.empty((B, T, S), np.float32)
    for b, r in enumerate(res.results):
        # outS[p, g, s, t'] = scoresT[s*128+p, g*1024+t']
        st = r["outS"].astype(np.float32)
        out[b] = st.transpose(2, 0, 1, 3).reshape(S, T).T
    return out, res


def kernel(xs_pad, spk_emb):
    out, _ = run({"xs_pad": xs_pad, "spk_emb": spk_emb})
    return out
